# revision 1
# baseline (speedup 1.0000x reference)
"""Trainium2 Bass kernel for nn_CTSimGLM: GLM spike-train simulation.

Algorithm (per core, uniform SPMD program; per-core variation only via input
data):
  1. stage 1: partial spatial projection  spat_i[b,t] = sum_{p in shard} sf[p]*stim[b,p,t]
     (P sharded 8 ways, 512 pixels/core; fp16 operands, fp32 PSUM accumulate,
     matmuls hidden under the stim DMA).
  2. partial gensig[t_out, b] = conv(spat_i, timecourse) + conv(coupled spikes
     for this core's 3 channels, coupling filters) + bias (core 0 only), all
     as Toeplitz-matrix matmuls accumulated in PSUM (fp32); the stim-independent
     coupling part completes during the DMA phase. The partial is transposed
     to [b, t_out] before the collective.
  3. AllGather of the 8 partial gensigs (28 KB each, cheaper than AllReduce's
     1.875x fabric pricing), then reduced over cores with one rank-32
     indicator matmul per 512-column block.
  4. Jacobi fixed-point iteration for the sigmoid autoregression, replicated on
     every core over all 128 (b, r) lanes: X <- sigmoid(G + F_fb @ X), with X
     the full spike raster laid out time-major ([128 t-local, chunk*lane],
     chunk c row i <-> t = 128c - 6 + i). 8 fp16 sweeps (first from X=0 needs
     only the G term) + 1 fp32 polish sweep reach ~3e-4 max-abs error vs the
     fp32 reference.
  5. PE-transpose X to lane-major, two 1024-column DMAs; host slices cols
     [6:2006].
"""

import os
from contextlib import ExitStack

import numpy as np

import concourse.bass as bass
import concourse.bacc as bacc
import concourse.tile as tile
import concourse.mybir as mybir
from concourse.bass_utils import run_bass_kernel_spmd
from concourse.masks import make_identity

ts = bass.ts

B, P, T, K, C, R = 4, 4096, 2000, 250, 24, 32
NCORES = 8
PSH = P // NCORES            # 512 pixels per core
PCH = PSH // 128             # 4 pixel chunks per core
CCH = C // NCORES            # 3 coupling channels per core
NT = 14                      # gensig/out tiles of 128 (covers t_out < 1792)
NCH = 16                     # X chunks ([128, 128] each, t = 128*c - 6 + i)
NW = 4                       # stage-1 time windows of 500
WW = 500
N_F16 = 8                    # fp16 Jacobi sweeps
N_FP32 = 1                   # fp32 polish sweeps

F32 = mybir.dt.float32
F16 = mybir.dt.float16
SIG = mybir.ActivationFunctionType.Sigmoid


def _toeplitz(filt, shift):
    """3 stacked [128,128] tiles: F_d[i, jj] = filt[128*d + shift + i - jj]."""
    i = np.arange(128)[:, None]
    jj = np.arange(128)[None, :]
    out = np.zeros((3, 128, 128), np.float32)
    for d in range(3):
        idx = 128 * d + shift + i - jj
        valid = (idx >= 0) & (idx < K)
        out[d] = np.where(valid, filt[np.clip(idx, 0, K - 1)], 0.0)
    return out


def _build_nc():
    nc = bacc.Bacc(
        "TRN2", target_bir_lowering=False, debug=False, num_devices=NCORES
    )

    stim_d = nc.dram_tensor("stim_sl", [B, PSH, T], F16, kind="ExternalInput")
    sf_d = nc.dram_tensor("sf_sl", [PCH, 128, 1], F16, kind="ExternalInput")
    cspk_d = nc.dram_tensor("cspk_t", [NCH * 128, CCH * B], F32, kind="ExternalInput")
    coupT_d = nc.dram_tensor("coupT", [CCH, 3, 128, 128], F32, kind="ExternalInput")
    tcT7_d = nc.dram_tensor("tcT7", [6, 128, 512], F16, kind="ExternalInput")
    fbT32_d = nc.dram_tensor("fbT32", [3, 128, 128], F32, kind="ExternalInput")
    fbTh_d = nc.dram_tensor("fbTh", [3, 128, 128], F16, kind="ExternalInput")
    indh_d = nc.dram_tensor("indh", [4, 128], F16, kind="ExternalInput")
    ind32_d = nc.dram_tensor("ind32", [4, 128], F32, kind="ExternalInput")
    ind328_d = nc.dram_tensor("ind328", [4 * NCORES, 4], F32, kind="ExternalInput")
    bias4_d = nc.dram_tensor("bias4", [1, 4], F32, kind="ExternalInput")
    x0_d = nc.dram_tensor("x0", [128, 256], F32, kind="ExternalInput")
    out_d = nc.dram_tensor("out_x", [128, NCH * 128], F32, kind="ExternalOutput")

    with tile.TileContext(nc) as tc, ExitStack() as ctx:
        consts = ctx.enter_context(tc.tile_pool(name="consts", bufs=1))
        dram = ctx.enter_context(tc.tile_pool(name="dram", bufs=1, space="DRAM"))

        # ---- stage 1 first: keep the big stim stream at the head of the
        # HWDGE queue; everything else loads via the Pool (SWDGE) queue.
        sf_s = consts.tile([128, PCH], F16)
        for pc in range(PCH):
            nc.sync.dma_start(sf_s[:, pc : pc + 1], sf_d[pc])

        partb_t = dram.tile([4, NT * 128], F32)
        gathb_t = dram.tile([NCORES * 4, NT * 128], F32, addr_space="Shared")
        with (
            tc.tile_pool(name="stim", bufs=8) as stim_pool,
            tc.tile_pool(name="psum_sp", bufs=4, space="PSUM") as psum_sp,
            tc.tile_pool(name="psum_g", bufs=2, space="PSUM") as psum_g,
            tc.tile_pool(name="psum_pgt", bufs=2, space="PSUM") as psum_pgt,
        ):
            # full [128, 2000] rows per DMA: 4 KB descriptors (2 KB+ needed to
            # avoid the per-descriptor floor); pc-outer matmuls accumulate into
            # 4 window groups packed at 32-aligned partitions of one psum bank
            # so only 4 matmuls trail the last DMA of each b
            spat_t = consts.tile([128, NCH * B], F16)  # time-major, unshifted chunks
            nc.vector.memset(spat_t[:], 0.0)
            # time-major spat directly from the matvec: stationary = stim
            # time-slices [128 p, 128 t], moving = sf column -> out [t, 1];
            # one psum bank per 4-chunk group, 4 groups in flight
            spt_view = None
            for b in range(B):
                sts = []
                for pc in range(PCH):
                    st = stim_pool.tile([128, T], F16, tag="st", name=f"st{b}{pc}")
                    eng = nc.sync if pc % 2 == 0 else nc.scalar
                    eng.dma_start(st[:], stim_d[b, ts(pc, 128), :])
                    sts.append(st)
                for ttg in range(4):
                    pst = psum_sp.tile([128, 4], F32, tag="sp")
                    for tt in range(4):
                        c = ttg * 4 + tt
                        hh = 128 if c < NCH - 1 else 80
                        for pc in range(PCH):
                            nc.tensor.matmul(
                                pst[0:hh, tt : tt + 1],
                                lhsT=sts[pc][:, 128 * c : 128 * c + hh],
                                rhs=sf_s[:, pc : pc + 1],
                                start=(pc == 0),
                                stop=(pc == PCH - 1),
                            )
                    dst = spat_t[:].rearrange("i (c b) -> i c b", b=B)
                    if ttg < 3:
                        nc.vector.tensor_copy(dst[:, ts(ttg, 4), b], pst[:, 0:4])
                    else:
                        nc.vector.tensor_copy(dst[:, 12:15, b], pst[:, 0:3])
                        nc.vector.tensor_copy(dst[0:80, 15:16, b], pst[0:80, 3:4])

            # ---- constants (Pool/SWDGE queue; overlap the stim stream) ----
            ident = consts.tile([128, 128], F32)
            make_identity(nc, ident)
            ones_row = consts.tile([1, 128], F32)
            nc.vector.memset(ones_row[:], 1.0)
            # preload the sigmoid table set during the stim DMA phase
            sigwarm = consts.tile([1, 1], F32)
            nc.vector.memset(sigwarm[:], 0.0)
            nc.scalar.activation(sigwarm[:], sigwarm[:], SIG)

            coupT_s = consts.tile([128, CCH * 3 * 128], F32)
            nc.gpsimd.dma_start(
                coupT_s[:].rearrange("i (c d j) -> i c d j", c=CCH, d=3),
                coupT_d[:].transpose([2, 0, 1, 3]),
            )
            cspk_s = consts.tile([128, NCH * CCH * B], F32)
            nc.gpsimd.dma_start(
                cspk_s[:].rearrange("i (c w) -> i c w", c=NCH),
                cspk_d[:].rearrange("(c i) w -> c i w", i=128).transpose([1, 0, 2]),
            )
            bias4_s = consts.tile([1, 4], F32)
            nc.gpsimd.dma_start(bias4_s[:], bias4_d[:])
            ind328_s = consts.tile([4 * NCORES, 4], F32)
            nc.gpsimd.dma_start(ind328_s[:], ind328_d[:])
            indh_s = consts.tile([4, 128], F16)
            nc.gpsimd.dma_start(indh_s[:], indh_d[:])
            fbTh_s = consts.tile([128, 3 * 128], F16)
            nc.gpsimd.dma_start(
                fbTh_s[:].rearrange("i (d j) -> i d j", d=3),
                fbTh_d[:].transpose([1, 0, 2]),
            )
            tcT7_s = consts.tile([128, 6 * 512], F16)
            nc.gpsimd.dma_start(
                tcT7_s[:].rearrange("i (r j) -> i r j", r=6),
                tcT7_d[:].transpose([1, 0, 2]),
            )
            fbT32_s = consts.tile([128, 3 * 128], F32)
            nc.gpsimd.dma_start(
                fbT32_s[:].rearrange("i (d j) -> i d j", d=3),
                fbT32_d[:].transpose([1, 0, 2]),
            )
            ind32_s = consts.tile([4, 128], F32)
            nc.gpsimd.dma_start(ind32_s[:], ind32_d[:])

            g_sb = consts.tile([128, NT * B], F32)   # gensig [t_out-local, tile*b]
            partGtA = consts.tile([4, NT * 128], F32)  # coup+bias partial [b, t_out]
            partGtB = consts.tile([4, NT * 128], F32)  # timecourse partial [b, t_out]
            Gt32 = consts.tile([4, NT * 128], F32)   # gensig [b, t_out]
            Gth = consts.tile([4, NT * 128], F16)

            # X rasters, double buffered (pure Jacobi: read one, write other)
            xah = consts.tile([128, NCH * 128], F16)
            xbh = consts.tile([128, NCH * 128], F16)
            xa32 = consts.tile([128, NCH * 128], F32)
            xb32 = consts.tile([128, NCH * 128], F32)
            nc.vector.memset(xah[:], 0.0)
            nc.vector.memset(xbh[:], 0.0)
            nc.vector.memset(xa32[:], 0.0)
            nc.vector.memset(xb32[:], 0.0)
            # initial spike window occupies exactly chunks 0..1
            nc.gpsimd.dma_start(xa32[:, 0:256], x0_d[:])
            nc.vector.tensor_copy(xb32[:, 0:256], xa32[:, 0:256])
            nc.vector.tensor_copy(xah[:, 0:256], xa32[:, 0:256])
            nc.vector.tensor_copy(xbh[:, 0:256], xa32[:, 0:256])

            # ---- partial gensig, split so its collective can start early:
            # part A (coupling + bias) is independent of stim and is gathered
            # DURING the DMA phase; part B (timecourse) trails the last stim
            # byte with only 3 matmuls + transpose per block ----
            for blk in range(4):
                njt = min(4, NT - blk * 4)
                pg = psum_g.tile([128, 16], F32, tag="pg", name=f"pgc{blk}")
                for q in range(njt):
                    j = blk * 4 + q
                    sl = pg[:, ts(q, 4)]
                    for cc in range(CCH):
                        for d in range(3):
                            col = (j + d) * (CCH * B) + cc * B
                            nc.tensor.matmul(
                                sl,
                                lhsT=coupT_s[:, ts(cc * 3 + d, 128)],
                                rhs=cspk_s[:, col : col + B],
                                start=(cc == 0 and d == 0),
                                stop=False,
                            )
                    nc.tensor.matmul(
                        sl,
                        lhsT=ones_row[0:1, :],
                        rhs=bias4_s[0:1, :],
                        start=False,
                        stop=True,
                    )
                nc.vector.tensor_copy(
                    g_sb[:, blk * 16 : blk * 16 + njt * 4], pg[:, 0 : njt * 4]
                )
                w = njt * 128
                pgt = psum_pgt.tile([4, 512], F32, tag="pgt", name=f"pgta{blk}")
                for q in range(njt):
                    nc.tensor.transpose(
                        pgt[0:4, ts(q, 128)],
                        g_sb[:, blk * 16 + q * 4 : blk * 16 + (q + 1) * 4],
                        ident[:],
                    )
                nc.vector.tensor_copy(
                    partGtA[0:4, blk * 512 : blk * 512 + w], pgt[0:4, 0:w]
                )
            # timecourse conv computed directly in [b, t_out] layout:
            # stationary = fp16 spat chunk [128 t_in, 4 b], moving = fp16
            # Toeplitz tile F_r[i, jj] = tc[128*(c - 4*blk) + i - jj]
            for blk in range(4):
                w = 512 if blk < 3 else 256
                pgt = psum_pgt.tile([4, 512], F32, tag="pgt", name=f"pgtb{blk}")
                chunks = [c for c in range(4 * blk, 4 * blk + 6) if c < NCH]
                for ci, c in enumerate(chunks):
                    r = c - 4 * blk
                    nc.tensor.matmul(
                        pgt[0:4, 0:w],
                        lhsT=spat_t[:, ts(c, 4)],
                        rhs=tcT7_s[:, r * 512 : r * 512 + w],
                        start=(ci == 0),
                        stop=(ci == len(chunks) - 1),
                    )
                nc.vector.tensor_add(
                    partGtB[0:4, blk * 512 : blk * 512 + w],
                    pgt[0:4, 0:w],
                    partGtA[0:4, blk * 512 : blk * 512 + w],
                )

        # ---- collective B (timecourse partials). The A reduction runs during
        # B's fabric time; the post-B chain is one matmul + two adds per block
        # (blocks align 1:1 with the sweep banks) ----
        nc.scalar.dma_start(partb_t[:], partGtB[:])
        nc.gpsimd.collective_compute(
            "AllGather",
            mybir.AluOpType.bypass,
            replica_groups=[list(range(NCORES))],
            ins=[partb_t.opt()],
            outs=[gathb_t.opt()],
        )
        gathb_s = consts.tile([NCORES * 4, NT * 128], F32)
        nc.gpsimd.dma_start(gathb_s[:], gathb_t[:])
        with tc.tile_pool(name="psum_gs", bufs=4, space="PSUM") as psum_gs:
            pgs_tiles = []
            for bank in range(4):
                w = min(512, NT * 128 - bank * 512)
                pgs = psum_gs.tile([4, 512], F32, tag="gs", name=f"pgs{bank}")
                nc.tensor.matmul(
                    pgs[0:4, 0:w],
                    lhsT=ind328_s[:],
                    rhs=gathb_s[:, bank * 512 : bank * 512 + w],
                    start=True,
                    stop=True,
                )
                nc.vector.tensor_copy(
                    Gth[:, bank * 512 : bank * 512 + w], pgs[0:4, 0:w]
                )
                pgs_tiles.append((pgs, w))
            # Gt32 feeds only the polish sweep much later
            for bank, (pgs, w) in enumerate(pgs_tiles):
                nc.vector.tensor_copy(
                    Gt32[:, bank * 512 : bank * 512 + w], pgs[0:4, 0:w]
                )

        # ---- Jacobi sweeps ----
        def sweep(src, dst, fb_s, gt_s, ind_s, psum_x):
            # bank 0..2: 4 out-tiles each; bank 3: tiles 12..13.
            # G-MMs open the accumulation (start) per 128-col slice; the three
            # batched shift matmuls (ap 512) then accumulate F_d.T @ X[chunks
            # bank*4+d ..] over the whole bank; src=None skips them (X = 0).
            for bank in range(4):
                ntile = 4 if bank < 3 else 2
                w = ntile * 128
                px = psum_x.tile([128, 512], F32, tag="px")
                if src is None:
                    for q in range(ntile):
                        j = bank * 4 + q
                        nc.tensor.matmul(
                            px[:, ts(q, 128)],
                            lhsT=gt_s[:, ts(j, 128)],
                            rhs=ind_s[:],
                            start=True,
                            stop=True,
                        )
                else:
                    nc.tensor.matmul(
                        px[:, 0:w],
                        lhsT=fb_s[:, 0:128],
                        rhs=src[:, (bank * 4) * 128 : (bank * 4) * 128 + w],
                        start=True,
                        stop=False,
                    )
                    for q in range(ntile):
                        j = bank * 4 + q
                        nc.tensor.matmul(
                            px[:, ts(q, 128)],
                            lhsT=gt_s[:, ts(j, 128)],
                            rhs=ind_s[:],
                            start=False,
                            stop=False,
                            skip_group_check=True,
                        )
                    nc.tensor.matmul(
                        px[:, 0:w],
                        lhsT=fb_s[:, 128:256],
                        rhs=src[:, (bank * 4 + 1) * 128 : (bank * 4 + 1) * 128 + w],
                        start=False,
                        stop=False,
                    )
                    nc.tensor.matmul(
                        px[:, 0:w],
                        lhsT=fb_s[:, 256:384],
                        rhs=src[:, (bank * 4 + 2) * 128 : (bank * 4 + 2) * 128 + w],
                        start=False,
                        stop=True,
                    )
                # ACT writes chunks bank*4+2.. (contiguous cols); garbage rows
                # in chunk 15 (t >= 2000) only feed garbage rows, host slices off
                nc.scalar.activation(
                    dst[:, (bank * 4 + 2) * 128 : (bank * 4 + 2) * 128 + w],
                    px[:, 0:w],
                    SIG,
                )

        with tc.tile_pool(name="psum_x", bufs=4, space="PSUM") as psum_x:
            # sweep 1 from X=0 needs no F matmuls; last fp16 sweep writes the
            # fp32 buffer directly (skips a separate promote copy)
            sweep(None, xah, fbTh_s, Gth, indh_s, psum_x)
            cur, nxt = xah, xbh
            for _ in range(N_F16 - 2):
                sweep(cur, nxt, fbTh_s, Gth, indh_s, psum_x)
                cur, nxt = nxt, cur
            sweep(cur, xa32, fbTh_s, Gth, indh_s, psum_x)
            cur32, nxt32 = xa32, xb32
            for _ in range(N_FP32):
                sweep(cur32, nxt32, fbT32_s, Gt32, ind32_s, psum_x)
                cur32, nxt32 = nxt32, cur32

        # ---- transpose to lane-major, write out in two 8-chunk groups
        # (4 KB descriptors hit the DMA floor exactly) ----
        with (
            tc.tile_pool(name="psum_o", bufs=4, space="PSUM") as psum_o,
            tc.tile_pool(name="ostage", bufs=2) as ostage,
        ):
            for g in range(2):
                so = ostage.tile([128, 1024], F32, tag="so")
                for half in range(2):
                    po = psum_o.tile([128, 512], F32, tag="po")
                    for q in range(4):
                        c = g * 8 + half * 4 + q
                        nc.tensor.transpose(
                            po[:, ts(q, 128)], cur32[:, ts(c, 128)], ident[:]
                        )
                    nc.vector.tensor_copy(so[:, ts(half, 512)], po[:])
                nc.sync.dma_start(out_d[:, ts(g, 1024)], so[:])

    nc.compile()
    return nc


_NC_CACHE = None


def _get_nc():
    global _NC_CACHE
    if _NC_CACHE is None:
        _NC_CACHE = _build_nc()
    return _NC_CACHE


def make_in_maps(
    stim_movie,
    initial_spike_section,
    coupled_cell_spikes,
    spatial_filter,
    timecourse_filter,
    feedback_filter,
    coupling_filters,
    bias,
):
    fbT = _toeplitz(feedback_filter, -6)
    # moving TC-Toeplitz: tcT7[r, i, jj] = tc[128*r + i - jj]
    i = np.arange(128)[:, None]
    jj = np.arange(512)[None, :]
    tcT7 = np.zeros((6, 128, 512), np.float32)
    for r in range(6):
        idx = 128 * r + i - jj
        valid = (idx >= 0) & (idx < K)
        tcT7[r] = np.where(valid, timecourse_filter[np.clip(idx, 0, K - 1)], 0.0)
    tcT7 = tcT7.astype(np.float16)
    lane_b = np.arange(128) // R  # lane = b*32 + r

    # initial window raster: chunks 0..1, X0[i, 128c+lane] = init[b(lane), 128c-6+i]
    x0 = np.zeros((128, 256), np.float32)
    for c in range(2):
        t = 128 * c - 6 + np.arange(128)
        valid = (t >= 0) & (t < K)
        x0[valid, 128 * c : 128 * (c + 1)] = initial_spike_section[
            lane_b[None, :], t[valid, None]
        ].astype(np.float32)

    ind = (lane_b[None, :] == np.arange(4)[:, None]).astype(np.float32)
    # gathered partial rows are (core r, b'): row r*4 + b' contributes to b=b'
    ind328 = np.tile(np.eye(4, dtype=np.float32), (NCORES, 1))
    stim_h = stim_movie.astype(np.float16)
    sf_h = spatial_filter.astype(np.float16)

    in_maps = []
    for core in range(NCORES):
        psl = slice(PSH * core, PSH * (core + 1))
        csl = slice(CCH * core, CCH * (core + 1))
        cspk_t = np.zeros((NCH * 128, CCH * B), np.float32)
        cspk_t[:T] = (
            coupled_cell_spikes[:, csl, :].transpose(2, 1, 0).reshape(T, CCH * B)
        )
        coupT = np.stack(
            [_toeplitz(coupling_filters[ch], 0) for ch in range(csl.start, csl.stop)]
        )
        bias4 = np.full((1, 4), np.float32(bias[0]) if core == 0 else 0.0, np.float32)
        in_maps.append(
            {
                "stim_sl": np.ascontiguousarray(stim_h[:, psl, :]),
                "sf_sl": np.ascontiguousarray(sf_h[psl].reshape(PCH, 128, 1)),
                "cspk_t": cspk_t,
                "coupT": coupT.astype(np.float32),
                "tcT7": tcT7,
                "fbT32": fbT,
                "fbTh": fbT.astype(np.float16),
                "indh": ind.astype(np.float16),
                "ind32": ind,
                "ind328": ind328,
                "bias4": bias4,
                "x0": x0,
            }
        )
    return in_maps


def kernel(**inputs):
    assert int(inputs["n_repeats"]) == R
    in_maps = make_in_maps(
        np.asarray(inputs["stim_movie"], np.float32),
        np.asarray(inputs["initial_spike_section"], np.float32),
        np.asarray(inputs["coupled_cell_spikes"], np.float32),
        np.asarray(inputs["spatial_filter"], np.float32),
        np.asarray(inputs["timecourse_filter"], np.float32),
        np.asarray(inputs["feedback_filter"], np.float32),
        np.asarray(inputs["coupling_filters"], np.float32),
        np.asarray(inputs["bias"], np.float32),
    )
    nc = _get_nc()
    res = run_bass_kernel_spmd(
        nc,
        in_maps,
        core_ids=list(range(NCORES)),
        trace=bool(int(os.environ.get("KERNEL_TRACE", "0"))),
    )
    out_x = res.results[0]["out_x"]  # (128, 2048), cols = t + 6
    out = np.ascontiguousarray(out_x[:, 6 : 6 + T]).reshape(B, R, T)
    kernel.last_results = res
    return out.astype(np.float32)

